# revision 9
# baseline (speedup 1.0000x reference)
"""Trainium2 Bass kernel for the KNet-style recurrent chain (batch=1).

Strategy (memory-bound, ~353MB fp32 weights -> ~177MB bf16):
  - ALL weights are pre-packed host-side into the exact SBUF layout and
    cast to bf16 (tolerance is 2e-2; bf16 end-to-end error ~4e-3), so
    every DMA is a contiguous copy with >=2KB per-partition lines.
  - The small GRU chain + small FCs are REPLICATED on all 8 cores
    (17.4MB bf16/core); FC2 is tensor-parallel: core k owns rows
    [5760k,5760k+5760) of W2a and the matching columns of W2b, computes
    a partial y[576]; the host sums the 8 partials and adds b2b.
  - Matvecs run activation-stationary on the PE: out[1,N] += x[K,1].T @
    Wt[K,N], weights moving in bf16 (1 col/cycle = 614GB/s consumption,
    faster than the 360GB/s HBM feed -> DMA-bound by design).
  - GRU rows are permuted host-side into NB=4 blocks of 144 hidden
    indices x 3 gates (r,z,n), so each block's gi/gh psum is [1,432]
    (one PSUM bank) and the gate combine runs per-block, keeping PSUM
    within 8 banks.
  - FC2a output stripes [1,512] get bias+relu then PE-transpose into the
    [128,45] bf16 stationary for FC2b, whose 45 K-chunk matmuls
    accumulate y in 2 PSUM banks (512+64) interleaved with FC2a so the
    kernel ends ~1us after the last W2b DMA.

If SHARD_GRUS is True the GRU matvecs are 8-way sharded (72 hidden
indices per core) and the three hidden states are exchanged with
on-device AllGather collectives (plus a dummy AllGather at t=0 to
absorb the first-collective barrier). Measured: the first collective
pays a ~50-70us cross-core rendezvous, so this is off by default.
"""

import sys

sys.path.insert(0, "/opt/trn_rl_repo")

import numpy as np
import ml_dtypes

BF16 = ml_dtypes.bfloat16
F32 = np.float32

NCORES = 8
H = 576                       # hidden size of all three GRUs
D2_IN, D2_HID, D2_OUT = 1152, 46080, 576
MSH = D2_HID // NCORES        # 5760 rows of W2a per core
STRIPES = [(n0, min(512, MSH - n0)) for n0 in range(0, MSH, 512)]
NKB = MSH // 128              # 45 FC2b K chunks per core
W2B_GRP = 5                   # FC2b K chunks per DMA group
W2B_GROUPS = [(g0, min(W2B_GRP, NKB - g0)) for g0 in range(0, NKB, W2B_GRP)]

SHARD_GRUS = False
if SHARD_GRUS:
    NB, BS = 1, H // NCORES   # one block of 72 owned indices per core
else:
    NB, BS = 4, 144           # four blocks x 144 indices, replicated
BS3 = 3 * BS

# stationary x-segment lists (the moving weight chunking mirrors these);
# the r,z gates run as ONE matvec over concat(h, x) with combined weight
# [Whh_rz | Wih_rz] so the PE accumulates gi_rz+gh_rz in PSUM (a DVE op
# may read only one PSUM operand, so gi+gh can't be added on the DVE)
XSEGS = {"q": [480], "sig": [H, 480], "s": [H, 960]}


def _nch(segs):
    return sum((L + 127) // 128 for L in segs)


# consts tensor layout (f32, [1, NCONST]); b2a is its own tensor
CONST_LAYOUT = []
for _g in ("q", "sig", "s"):
    CONST_LAYOUT += [(f"h_own_{_g}", NB * BS), (f"brz_{_g}", NB * 2 * BS),
                     (f"bin_{_g}", NB * BS), (f"bhn_{_g}", NB * BS)]
CONST_LAYOUT += [("b5", 480), ("b6", 480), ("b7", 960), ("b1", H)]
CONST_OFF = {}
_o = 0
for _n, _l in CONST_LAYOUT:
    CONST_OFF[_n] = _o
    _o += _l
NCONST = _o

_CACHE = {}


def _build_program():
    import concourse.bass as bass  # noqa: F401
    from concourse import bacc, mybir
    import concourse.tile as tile

    f32 = mybir.dt.float32
    bf16 = mybir.dt.bfloat16
    AF = mybir.ActivationFunctionType

    nc = bacc.Bacc("TRN2", target_bir_lowering=False, debug=False,
                   num_devices=NCORES)

    def din(name, shape, dt):
        return nc.dram_tensor(name, list(shape), dt, kind="ExternalInput")

    d_consts = din("consts", (1, NCONST), f32)
    d_b2a = din("b2a", (1, MSH), f32)
    d_stat = din("stat", (128, 18), bf16)

    wshapes = {"w5": (24, 480), "w6": (24, 480), "w7": (48, 960),
               "w1": (128, 5 * H)}
    for g in ("q", "sig", "s"):
        nch_x, nch_h = _nch(XSEGS[g]), _nch([H])
        wshapes[f"wrz_{g}"] = (128, NB * (nch_h + nch_x) * 2 * BS)
        wshapes[f"whn_{g}"] = (128, NB * nch_h * BS)
        wshapes[f"win_{g}"] = (128, NB * nch_x * BS)
    dw = {k: din(k, v, bf16) for k, v in wshapes.items()}
    d_w2a = din("w2a", (128, 9 * MSH), bf16)
    d_w2b = din("w2b", (128, NKB * D2_OUT), bf16)
    d_y = nc.dram_tensor("y", [1, D2_OUT], f32, kind="ExternalOutput")
    groups = [list(range(NCORES))]

    with tile.TileContext(nc) as tc:
        with (
            tc.tile_pool(name="const", bufs=1) as constp,
            tc.tile_pool(name="vecs", bufs=1) as vecp,
            tc.tile_pool(name="smallw", bufs=1) as swp,
            tc.tile_pool(name="bigw", bufs=1) as bigp,
            tc.tile_pool(name="ps", bufs=1, space="PSUM") as psp,
            tc.tile_pool(name="dram", bufs=1, space="DRAM") as dramp,
        ):
            # ---------------- DMA phase (sync queue, consumption order)
            consts = constp.tile([1, NCONST], f32, name="consts",
                                 tag="consts")
            nc.sync.dma_start(out=consts, in_=d_consts[:])
            stat = constp.tile([128, 18], bf16, name="stat", tag="stat")
            nc.sync.dma_start(out=stat, in_=d_stat[:])

            def cslice(nm, o, ln):
                return consts[0:1, CONST_OFF[nm] + o: CONST_OFF[nm] + o + ln]

            def wtile(key, shape, bufs=4):
                t = swp.tile(list(shape), bf16, name=key, tag="sw",
                             bufs=bufs)
                return t

            def wdma(t, key, c0, cn):
                nc.sync.dma_start(out=t, in_=dw[key][:, c0:c0 + cn])

            # small weights, in chain-consumption order
            sw = {}

            def gru_wdma(g):
                nch_x, nch_h = _nch(XSEGS[g]), _nch([H])
                cols = {"wrz": (nch_h + nch_x) * 2 * BS,
                        "whn": nch_h * BS, "win": nch_x * BS}
                for b in range(NB):
                    for kind in ("wrz", "whn", "win"):
                        ncol = cols[kind]
                        t = wtile(f"{kind}_{g}_{b}", (128, ncol))
                        wdma(t, f"{kind}_{g}", b * ncol, ncol)
                        sw[f"{kind}_{g}_{b}"] = t

            sw["w5"] = swp.tile([24, 480], bf16, name="w5", tag="w5")
            wdma(sw["w5"], "w5", 0, 480)
            gru_wdma("q")
            sw["w6"] = swp.tile([24, 480], bf16, name="w6", tag="w6")
            wdma(sw["w6"], "w6", 0, 480)
            gru_wdma("sig")
            sw["w7"] = swp.tile([48, 960], bf16, name="w7", tag="w7")
            wdma(sw["w7"], "w7", 0, 960)
            sw["w1"] = swp.tile([128, 5 * H], bf16, name="w1", tag="w1")
            wdma(sw["w1"], "w1", 0, 5 * H)
            gru_wdma("s")
            b2a = constp.tile([1, MSH], f32, name="b2a", tag="b2a")
            nc.sync.dma_start(out=b2a, in_=d_b2a[:])

            # big weights: W2a stripes and W2b groups, interleaved in the
            # order the FC2 loop consumes them
            w2a_t, w2b_t = [], []
            gi_iter = iter(range(len(W2B_GROUPS)))
            for si, (n0, nsz) in enumerate(STRIPES):
                t = bigp.tile([128, 9 * 512], bf16, name=f"w2a_{si}",
                              tag="w2a", bufs=3)
                nc.sync.dma_start(out=t[0:128, 0:9 * nsz],
                                  in_=d_w2a[:, 9 * n0: 9 * n0 + 9 * nsz])
                w2a_t.append(t)
                gneed = min((n0 + nsz) // 128, NKB) - 1  # last chunk used
                while len(w2b_t) <= gneed // W2B_GRP:
                    gidx = next(gi_iter)
                    g0, gn = W2B_GROUPS[gidx]
                    tb = bigp.tile([128, W2B_GRP * D2_OUT], bf16,
                                   name=f"w2b_{gidx}", tag="w2b", bufs=3)
                    nc.sync.dma_start(
                        out=tb[0:128, 0:gn * D2_OUT],
                        in_=d_w2b[:, g0 * D2_OUT:(g0 + gn) * D2_OUT])
                    w2b_t.append(tb)

            # ---------------- helpers
            ident = constp.tile([1, 1], f32, name="ident", tag="ident")
            nc.vector.memset(ident, 1.0)

            def seg_chunks(segs_stat):
                """[(stat_tile, col0, d)...] -> [(ap, ksz)...]"""
                out = []
                for st, c0, d in segs_stat:
                    nbk = (d + 127) // 128
                    for c in range(nbk):
                        ksz = min(128, d - c * 128)
                        out.append((st[0:ksz, c0 + c: c0 + c + 1], ksz))
                return out

            def matvec(wt, wt_c0, segs_stat, M, psums):
                """psums: [(ps_tile, m0, msz)]; wt cols chunked by M."""
                chunks = seg_chunks(segs_stat)
                nchk = len(chunks)
                for ci, (ap, ksz) in enumerate(chunks):
                    for ps, m0, msz in psums:
                        nc.tensor.matmul(
                            ps[0:1, 0:msz], ap,
                            wt[0:ksz, wt_c0 + ci * M + m0:
                               wt_c0 + ci * M + m0 + msz],
                            start=(ci == 0), stop=(ci == nchk - 1),
                            skip_group_check=True)

            def mvp(nm, msz=512):
                return psp.tile([1, 512], f32, name=nm, tag="mvp", bufs=3)

            def to_stat(src_ap_tile, src_off, d, name):
                """free-layout f32 [1,d] slice -> bf16 P-layout [128,nc]"""
                n_m = (d + 127) // 128
                ps_t = psp.tile([128, 12], f32, name=f"pst_{name}",
                                tag="tp", bufs=1)
                for c in range(n_m):
                    csz = min(128, d - c * 128)
                    nc.tensor.matmul(
                        ps_t[0:csz, c:c + 1],
                        src_ap_tile[0:1, src_off + c * 128:
                                    src_off + c * 128 + csz],
                        ident, is_transpose=True,
                        start=(c == 0), stop=(c == n_m - 1),
                        skip_group_check=True)
                st = vecp.tile([128, n_m], bf16, name=name, tag=name)
                nc.vector.tensor_copy(st, ps_t[:, 0:n_m])
                return st

            def fc(wkey, segs_stat, M, bias, act, out_name):
                """small FC: relu(W @ x + b) in free layout [1, M]"""
                psums = []
                for m0 in range(0, M, 512):
                    msz = min(512, M - m0)
                    psums.append((mvp(f"ps_{out_name}_{m0}"), m0, msz))
                matvec(sw[wkey], 0, segs_stat, M, psums)
                outf = vecp.tile([1, M], f32, name=out_name, tag=out_name)
                for ps, m0, msz in psums:
                    nc.vector.tensor_add(outf[0:1, m0:m0 + msz],
                                         ps[0:1, 0:msz],
                                         cslice(bias, m0, msz))
                nc.scalar.activation(outf, outf, act)
                return outf

            def gru(g, segs_stat_x, h_stat_c0, dest_tile, dest_off):
                """per-block matvecs + gate combine; writes h' into
                dest_tile[0:1, dest_off:dest_off+NB*BS]"""
                segs_h = [(stat, h_stat_c0, H)]
                for b in range(NB):
                    # rz psum accumulates BOTH Whh_rz@h and Wih_rz@x
                    ps_rz = mvp(f"rz_{g}_{b}")
                    matvec(sw[f"wrz_{g}_{b}"], 0, segs_h + segs_stat_x,
                           2 * BS, [(ps_rz, 0, 2 * BS)])
                    ps_hn = mvp(f"hn_{g}_{b}")
                    matvec(sw[f"whn_{g}_{b}"], 0, segs_h, BS,
                           [(ps_hn, 0, BS)])
                    ps_in = mvp(f"in_{g}_{b}")
                    matvec(sw[f"win_{g}_{b}"], 0, segs_stat_x, BS,
                           [(ps_in, 0, BS)])
                    # r,z = sigmoid(psum_rz+brz) ; n = tanh(gi_n+bin +
                    #   r*(gh_n+bhn)) ; h' = n + z*(h-n)
                    t1 = vecp.tile([1, 2 * BS], f32, name=f"t1_{g}{b}",
                                   tag="t1", bufs=2)
                    nc.vector.tensor_add(t1, ps_rz[0:1, 0:2 * BS],
                                         cslice(f"brz_{g}", b * 2 * BS,
                                                2 * BS))
                    rz = vecp.tile([1, 2 * BS], f32, name=f"rz_{g}{b}",
                                   tag="rz", bufs=2)
                    nc.scalar.activation(rz, t1, AF.Sigmoid)
                    t2 = vecp.tile([1, BS], f32, name=f"t2_{g}{b}",
                                   tag="t2", bufs=2)
                    nc.vector.tensor_add(t2, ps_hn[0:1, 0:BS],
                                         cslice(f"bhn_{g}", b * BS, BS))
                    nc.vector.tensor_mul(t2, rz[0:1, 0:BS], t2)
                    t3 = vecp.tile([1, BS], f32, name=f"t3_{g}{b}",
                                   tag="t3", bufs=2)
                    nc.vector.tensor_add(t3, ps_in[0:1, 0:BS],
                                         cslice(f"bin_{g}", b * BS, BS))
                    nc.vector.tensor_add(t3, t3, t2)
                    n_t = vecp.tile([1, BS], f32, name=f"n_{g}{b}",
                                    tag="n_t", bufs=2)
                    nc.scalar.activation(n_t, t3, AF.Tanh)
                    t4 = vecp.tile([1, BS], f32, name=f"t4_{g}{b}",
                                   tag="t4", bufs=2)
                    nc.vector.tensor_sub(t4, cslice(f"h_own_{g}", b * BS,
                                                    BS), n_t)
                    nc.vector.tensor_mul(t4, rz[0:1, BS:2 * BS], t4)
                    nc.vector.tensor_add(
                        dest_tile[0:1, dest_off + b * BS:
                                  dest_off + (b + 1) * BS], n_t, t4)

            def allgather(idx, src_ap, dest_ap):
                cin = dramp.tile([1, BS], f32, name=f"ccin{idx}",
                                 tag=f"ccin{idx}")
                cout = dramp.tile([1, H], f32, name=f"ccout{idx}",
                                  tag=f"ccout{idx}")
                nc.scalar.dma_start(out=cin[:], in_=src_ap)
                nc.gpsimd.collective_compute(
                    "AllGather", mybir.AluOpType.bypass,
                    replica_groups=groups,
                    ins=[cin.opt()], outs=[cout.opt()])
                nc.scalar.dma_start(out=dest_ap, in_=cout[:])

            # ---------------- the chain
            in2_f = vecp.tile([1, D2_IN], f32, name="in2_f", tag="in2_f")
            hq_f = vecp.tile([1, H], f32, name="hq_f", tag="hq_f")

            if SHARD_GRUS:
                # dummy collective at t=0 absorbs the first-CC barrier
                dumm = vecp.tile([1, 8], f32, name="dumm", tag="dumm")
                nc.vector.memset(dumm, 0.0)
                din0 = dramp.tile([1, 8], f32, name="dccin", tag="dccin")
                dout0 = dramp.tile([1, 64], f32, name="dccout",
                                   tag="dccout")
                nc.scalar.dma_start(out=din0[:], in_=dumm)
                nc.gpsimd.collective_compute(
                    "AllGather", mybir.AluOpType.bypass,
                    replica_groups=groups,
                    ins=[din0.opt()], outs=[dout0.opt()])

            out5_f = fc("w5", [(stat, 0, 24)], 480, "b5", AF.Relu,
                        "out5_f")
            out5_st = to_stat(out5_f, 0, 480, "out5_st")

            if SHARD_GRUS:
                hq_sh = vecp.tile([1, BS], f32, name="hq_sh", tag="hq_sh")
                gru("q", [(out5_st, 0, 480)], 3, hq_sh, 0)
                allgather(0, hq_sh, hq_f[0:1, 0:H])
            else:
                gru("q", [(out5_st, 0, 480)], 3, hq_f, 0)
            hq_st = to_stat(hq_f, 0, H, "hq_st")

            out6_f = fc("w6", [(stat, 1, 24)], 480, "b6", AF.Relu,
                        "out6_f")
            out6_st = to_stat(out6_f, 0, 480, "out6_st")

            if SHARD_GRUS:
                hsig_sh = vecp.tile([1, BS], f32, name="hsig_sh",
                                    tag="hsig_sh")
                gru("sig", [(hq_st, 0, H), (out6_st, 0, 480)], 8,
                    hsig_sh, 0)
                allgather(1, hsig_sh, in2_f[0:1, 0:H])
            else:
                gru("sig", [(hq_st, 0, H), (out6_st, 0, 480)], 8,
                    in2_f, 0)
            hsig_st = to_stat(in2_f, 0, H, "hsig_st")

            out7_f = fc("w7", [(stat, 2, 48)], 960, "b7", AF.Relu,
                        "out7_f")
            out7_st = to_stat(out7_f, 0, 960, "out7_st")
            out1_f = fc("w1", [(hsig_st, 0, H)], H, "b1", AF.Relu,
                        "out1_f")
            out1_st = to_stat(out1_f, 0, H, "out1_st")

            if SHARD_GRUS:
                hs_sh = vecp.tile([1, BS], f32, name="hs_sh", tag="hs_sh")
                gru("s", [(out1_st, 0, H), (out7_st, 0, 960)], 13,
                    hs_sh, 0)
                allgather(2, hs_sh, in2_f[0:1, H:2 * H])
            else:
                gru("s", [(out1_st, 0, H), (out7_st, 0, 960)], 13,
                    in2_f, H)
            in2_st = to_stat(in2_f, 0, D2_IN, "in2_st")

            # ---------------- FC2a + FC2b, stripe-interleaved
            h_fc = constp.tile([128, NKB], bf16, name="h_fc", tag="h_fc")
            ps_y5 = psp.tile([1, 512], f32, name="ps_y5", tag="y5",
                             bufs=1)
            ps_y6 = psp.tile([1, 64], f32, name="ps_y6", tag="y6",
                             bufs=1)
            for si, (n0, nsz) in enumerate(STRIPES):
                wt = w2a_t[si]
                psf = psp.tile([1, 512], f32, name=f"psf{si}", tag="fca",
                               bufs=2)
                for bb in range(9):
                    nc.tensor.matmul(
                        psf[0:1, 0:nsz], in2_st[0:128, bb:bb + 1],
                        wt[0:128, bb * nsz:(bb + 1) * nsz],
                        start=(bb == 0), stop=(bb == 8),
                        skip_group_check=True)
                tf = vecp.tile([1, 512], f32, name=f"tf{si}", tag="tf",
                               bufs=2)
                nc.vector.tensor_add(tf[0:1, 0:nsz], psf[0:1, 0:nsz],
                                     b2a[0:1, n0:n0 + nsz])
                nc.scalar.activation(tf[0:1, 0:nsz], tf[0:1, 0:nsz],
                                     AF.Relu)
                nbk = nsz // 128
                ps_t = psp.tile([128, 12], f32, name=f"pstf{si}",
                                tag="tp", bufs=1)
                for c in range(nbk):
                    nc.tensor.matmul(
                        ps_t[0:128, c:c + 1],
                        tf[0:1, c * 128:(c + 1) * 128], ident,
                        is_transpose=True, start=(c == 0),
                        stop=(c == nbk - 1), skip_group_check=True)
                kb0 = n0 // 128
                nc.vector.tensor_copy(h_fc[:, kb0:kb0 + nbk],
                                      ps_t[:, 0:nbk])
                for kb in range(kb0, kb0 + nbk):
                    gg, jj = kb // W2B_GRP, kb % W2B_GRP
                    wbt = w2b_t[gg]
                    nc.tensor.matmul(
                        ps_y5[0:1, 0:512], h_fc[0:128, kb:kb + 1],
                        wbt[0:128, jj * D2_OUT:jj * D2_OUT + 512],
                        start=(kb == 0), stop=(kb == NKB - 1),
                        skip_group_check=True)
                    nc.tensor.matmul(
                        ps_y6[0:1, 0:64], h_fc[0:128, kb:kb + 1],
                        wbt[0:128, jj * D2_OUT + 512:jj * D2_OUT + 576],
                        start=(kb == 0), stop=(kb == NKB - 1),
                        skip_group_check=True)
            y_sb = vecp.tile([1, D2_OUT], f32, name="y_sb", tag="y_sb")
            nc.vector.tensor_copy(y_sb[0:1, 0:512], ps_y5)
            nc.vector.tensor_copy(y_sb[0:1, 512:576], ps_y6)
            nc.sync.dma_start(out=d_y[:], in_=y_sb)

    nc.compile()
    return nc


def _get_program():
    if "nc" not in _CACHE:
        _CACHE["nc"] = _build_program()
    return _CACHE["nc"]


# ----------------------------------------------------------------------------
# host-side data prep
# ----------------------------------------------------------------------------


def _packT(W, segs):
    """W [M,K] -> [128, nch*M] bf16; chunk b rows p = W[:, koff+128b+p].T,
    zero-padded partition tails."""
    M, K = W.shape
    assert sum(segs) == K, (segs, K)
    blocks = []
    off = 0
    for L in segs:
        for c0 in range(0, L, 128):
            ksz = min(128, L - c0)
            blk = np.zeros((128, M), F32)
            blk[0:ksz] = W[:, off + c0: off + c0 + ksz].T
            blocks.append(blk)
        off += L
    return np.ascontiguousarray(np.concatenate(blocks, axis=1)).astype(BF16)


def _gru_rows(so, b):
    """weight rows for block b: gates r,z,n x BS hidden indices."""
    return np.concatenate(
        [np.arange(g * H + so + b * BS, g * H + so + b * BS + BS)
         for g in range(3)])


def _play_cols(v):
    buf = np.zeros((5, 128), F32)
    buf.reshape(-1)[:H] = v
    return buf.T


def _prep_inputs(inputs):
    g = {k: np.asarray(v, F32) for k, v in inputs.items()}

    stat = np.zeros((128, 18), F32)
    stat[0:24, 0] = g["fw_evol_diff"]
    stat[0:24, 1] = g["fw_update_diff"]
    stat[0:48, 2] = np.concatenate([g["obs_diff"], g["obs_innov_diff"]])
    stat[:, 3:8] = _play_cols(g["h_Q"])
    stat[:, 8:13] = _play_cols(g["h_Sigma"])
    stat[:, 13:18] = _play_cols(g["h_S"])

    common = {
        "stat": stat.astype(BF16),
        "w5": np.ascontiguousarray(g["W5"].T).astype(BF16),
        "w6": np.ascontiguousarray(g["W6"].T).astype(BF16),
        "w7": np.ascontiguousarray(g["W7"].T).astype(BF16),
        "w1": _packT(g["W1"], [H]),
    }

    in_maps = []
    for k in range(NCORES):
        m = dict(common)
        so = BS * k if SHARD_GRUS else 0
        consts = np.zeros(NCONST, F32)

        def put(name, v):
            o = CONST_OFF[name]
            consts[o:o + len(v)] = v

        put("b5", g["b5"])
        put("b6", g["b6"])
        put("b7", g["b7"])
        put("b1", g["b1"])
        for tag, suf, hname in (("q", "Q", "h_Q"), ("sig", "Sig", "h_Sigma"),
                                ("s", "S", "h_S")):
            bih, bhh = g[f"bih_{suf}"], g[f"bhh_{suf}"]
            Wih, Whh = g[f"Wih_{suf}"], g[f"Whh_{suf}"]
            rzp, hnp, inp = [], [], []
            for b in range(NB):
                rows = _gru_rows(so, b)
                rz_rows, n_rows = rows[:2 * BS], rows[2 * BS:]
                put_o = b * 2 * BS
                consts[CONST_OFF[f"brz_{tag}"] + put_o:
                       CONST_OFF[f"brz_{tag}"] + put_o + 2 * BS] = \
                    (bih + bhh)[rz_rows]
                consts[CONST_OFF[f"bin_{tag}"] + b * BS:
                       CONST_OFF[f"bin_{tag}"] + (b + 1) * BS] = \
                    bih[n_rows]
                consts[CONST_OFF[f"bhn_{tag}"] + b * BS:
                       CONST_OFF[f"bhn_{tag}"] + (b + 1) * BS] = \
                    bhh[n_rows]
                wrz = np.concatenate([Whh[rz_rows], Wih[rz_rows]], axis=1)
                rzp.append(_packT(wrz, [H] + XSEGS[tag]))
                hnp.append(_packT(Whh[n_rows], [H]))
                inp.append(_packT(Wih[n_rows], XSEGS[tag]))
            consts[CONST_OFF[f"h_own_{tag}"]:
                   CONST_OFF[f"h_own_{tag}"] + NB * BS] = \
                g[hname][so:so + NB * BS]
            m[f"wrz_{tag}"] = np.concatenate(rzp, axis=1)
            m[f"whn_{tag}"] = np.concatenate(hnp, axis=1)
            m[f"win_{tag}"] = np.concatenate(inp, axis=1)
        m["consts"] = consts.reshape(1, -1)
        m["b2a"] = g["b2a"][k * MSH:(k + 1) * MSH].reshape(1, -1).copy()

        Wa = g["W2a"][k * MSH:(k + 1) * MSH]          # [5760, 1152]
        stripes = []
        for n0, nsz in STRIPES:
            blk = Wa[n0:n0 + nsz].reshape(nsz, 9, 128)
            stripes.append(blk.transpose(2, 1, 0).reshape(128, 9 * nsz))
        m["w2a"] = np.ascontiguousarray(
            np.concatenate(stripes, axis=1)).astype(BF16)
        Wb = g["W2b"][:, k * MSH:(k + 1) * MSH]       # [576, 5760]
        m["w2b"] = np.ascontiguousarray(
            Wb.reshape(D2_OUT, NKB, 128).transpose(2, 1, 0)
            .reshape(128, NKB * D2_OUT)).astype(BF16)
        in_maps.append(m)
    return in_maps


def run(trace=False, **inputs):
    from concourse.bass_utils import run_bass_kernel_spmd

    nc = _get_program()
    in_maps = _prep_inputs(inputs)
    res = run_bass_kernel_spmd(nc, in_maps, list(range(NCORES)), trace=trace)
    y = np.zeros(D2_OUT, np.float64)
    for r in res.results:
        y += r["y"].reshape(-1).astype(np.float64)
    out = (y.astype(F32) + np.asarray(inputs["b2b"], F32)).reshape(24, 24)
    return out, res


def kernel(**inputs):
    out, _ = run(trace=False, **inputs)
    return out
